# revision 1
# baseline (speedup 1.0000x reference)
"""MoE expert-parallel FFN kernel for Trainium2 (8 NeuronCores).

Problem: 8192 tokens, d_model=768, d_ff=3072, 8 experts; each token is
routed (local_eid) to one expert's FFN: y = (relu(x@W1[e]+b1[e])@W2[e]+b2[e])*gate.

Sharding: expert parallelism — core e gets expert e's weights plus the
(gathered, transposed, zero-padded) tokens routed to expert e. The device
computes, per core, a dense 2-layer FFN in transposed orientation:

    hT[d_ff, T]    = relu(W1.T @ xT + b1)      (lhsT=W1 natural layout)
    yT[d_model, T] = (W2.T @ hT + b2) * gate   (lhsT=W2 natural layout)

so both weights are consumed in their natural [K, M] layouts and biases land
on the partition dim (per-partition activation bias). Gate is broadcast
across partitions once per core. Host side does the gather (by local_eid)
and the scatter-back, which is the all-to-all dispatch of the sharding hint
performed at shard/unshard time.

Matmul operands are bf16 (PE streams 1 col/cycle for both fp32r and bf16, so
compute time is unchanged, but weight DMA bytes halve and FWL engages).
Accumulation stays fp32 in PSUM; rel err ~1e-3 vs the fp32 reference.
Weights are pre-arranged on the host into the SBUF tile layout
([p, group, k, n]) so each weight DMA is a single fully-contiguous
multi-KB-per-partition transfer instead of many 1KB strided lines.
"""

import numpy as np
import ml_dtypes

import concourse.bacc as bacc
import concourse.mybir as mybir
import concourse.tile as tile
from concourse.bass_utils import run_bass_kernel_spmd

P = 128
D_MODEL = 768
D_FF = 3072
N_EXPERTS = 8
N_CORES = 8
KM1 = D_MODEL // P   # 6  k-tiles for mm1
M1 = D_FF // P       # 24 m-tiles for mm1
KM2 = D_FF // P      # 24 k-tiles for mm2
M2 = D_MODEL // P    # 6  m-tiles for mm2
T_BLOCK_MAX = 1344   # max tokens per on-chip block (SBUF budget)
MG = 2               # W1 m-tiles per streamed weight group
F32 = mybir.dt.float32
BF16 = mybir.dt.bfloat16


def _chunks(T):
    """Split T into contiguous chunks, each <=512 (PSUM fp32 bank limit),
    >=256 where possible, and a multiple of 8 (32B DMA alignment).
    T must be a multiple of 8."""
    assert T % 8 == 0, T
    n = -(-T // 512)
    base = -(-(T // 8) // n) * 8
    out = []
    s = 0
    while s < T:
        e = min(s + base, T)
        out.append((s, e))
        s = e
    return out


def _emit(tc, aps, T, mmdt, reps=1, body_reps=1):
    nc = tc.nc
    Relu = mybir.ActivationFunctionType.Relu

    with (
        tc.tile_pool(name="const", bufs=1) as const,
        tc.tile_pool(name="xres", bufs=1) as xres,
        tc.tile_pool(name="hres", bufs=1) as hres,
        tc.tile_pool(name="w1s", bufs=2) as w1p,
        tc.tile_pool(name="w2s", bufs=2) as w2p,
        tc.tile_pool(name="ev", bufs=4) as evp,
        tc.tile_pool(name="ps", bufs=2, space="PSUM") as psp,
    ):
        b1_sb = const.tile([P, M1], F32)
        nc.sync.dma_start(out=b1_sb[:], in_=aps["b1"].rearrange("(m p) -> p m", p=P))
        b2_sb = const.tile([P, M2], F32)
        nc.sync.dma_start(out=b2_sb[:], in_=aps["b2"].rearrange("(m p) -> p m", p=P))

        import contextlib
        loop_cm = tc.For_i(0, reps, 1) if reps != 1 else contextlib.nullcontext()
        with loop_cm:
            for _ in range(body_reps):
                _emit_body(
                    tc, aps, T, mmdt,
                    const, xres, hres, w1p, w2p, evp, psp, b1_sb, b2_sb,
                )


def _emit_body(tc, aps, T, mmdt,
               const, xres, hres, w1p, w2p, evp, psp, b1_sb, b2_sb):
    nc = tc.nc
    xT, w1, w2, gate, yT = aps["xT"], aps["w1"], aps["w2"], aps["gate"], aps["yT"]
    Relu = mybir.ActivationFunctionType.Relu

    n_blocks = -(-T // T_BLOCK_MAX)
    TB = -(-(-(-T // n_blocks)) // 8) * 8  # per-block tokens, multiple of 8

    for blk in range(n_blocks):
        t0 = blk * TB
        t1 = min(T, t0 + TB)
        if t1 <= t0:
            continue
        Tb = t1 - t0
        chs = _chunks(Tb)

        x_all = xres.tile([P, KM1, Tb], mmdt, tag="x")
        for k in range(KM1):
            nc.sync.dma_start(out=x_all[:, k], in_=xT[:, k, t0:t1])
        hT = hres.tile([P, KM2, Tb], mmdt, tag="h")

        # ---- mm1: hT[:, m, :] = relu(W1[:, mP:(m+1)P].T @ xT + b1[m]) ----
        for mg in range(M1 // MG):
            wt = w1p.tile([P, KM1, MG * P], mmdt, tag="w1")
            nc.sync.dma_start(out=wt[:], in_=w1[:, mg])
            for ms in range(MG):
                m = mg * MG + ms
                pst = [
                    psp.tile([P, e - s], F32, tag=f"ps{ci}", name=f"ps{ci}")
                    for ci, (s, e) in enumerate(chs)
                ]
                for k in range(KM1):
                    for ci, (s, e) in enumerate(chs):
                        nc.tensor.matmul(
                            pst[ci][:],
                            lhsT=wt[:, k, ms * P:(ms + 1) * P],
                            rhs=x_all[:, k, s:e],
                            start=(k == 0),
                            stop=(k == KM1 - 1),
                        )
                for ci, (s, e) in enumerate(chs):
                    nc.scalar.activation(
                        hT[:, m, s:e], pst[ci][:], Relu, bias=b1_sb[:, m:m + 1]
                    )

        # ---- mm2: yT[:, m, :] = (W2[:, mP:(m+1)P].T @ hT + b2[m]) * gate ----
        # gate broadcast emitted late: only needed at mm2 evict, keeps the
        # startup DMA window clear for x/W1 (the PE-critical loads).
        gate_sb = xres.tile([P, Tb], F32, tag="g")
        nc.sync.dma_start(out=gate_sb[:], in_=gate[t0:t1].partition_broadcast(P))
        for m in range(M2):
            wt2 = w2p.tile([P, KM2, P], mmdt, tag="w2")
            nc.sync.dma_start(out=wt2[:], in_=w2[:, m])
            pst = [
                psp.tile([P, e - s], F32, tag=f"ps{ci}", name=f"ps{ci}")
                for ci, (s, e) in enumerate(chs)
            ]
            for k in range(KM2):
                for ci, (s, e) in enumerate(chs):
                    nc.tensor.matmul(
                        pst[ci][:],
                        lhsT=wt2[:, k, :],
                        rhs=hT[:, k, s:e],
                        start=(k == 0),
                        stop=(k == KM2 - 1),
                    )
            for ci, (s, e) in enumerate(chs):
                yt = evp.tile([P, e - s], F32, tag="y")
                nc.vector.tensor_scalar_add(yt[:], pst[ci][:], b2_sb[:, m:m + 1])
                nc.vector.tensor_mul(yt[:], yt[:], gate_sb[:, s:e])
                nc.sync.dma_start(out=yT[:, m, t0 + s:t0 + e], in_=yt[:])


def build_nc(T, mmdt=BF16, reps=1, body_reps=1):
    nc = bacc.Bacc("TRN2", target_bir_lowering=False, debug=False)
    NG1 = M1 // MG
    aps = {
        # xT[p, k, t] = x[t, k*P+p]
        "xT": nc.dram_tensor("xT", [P, KM1, T], mmdt, kind="ExternalInput").ap(),
        # w1[p, g, k, j] = W1[k*P+p, g*MG*P+j]
        "w1": nc.dram_tensor("w1", [P, NG1, KM1, MG * P], mmdt, kind="ExternalInput").ap(),
        "b1": nc.dram_tensor("b1", [D_FF], F32, kind="ExternalInput").ap(),
        # w2[p, m, k, n] = W2[k*P+p, m*P+n]
        "w2": nc.dram_tensor("w2", [P, M2, KM2, P], mmdt, kind="ExternalInput").ap(),
        "b2": nc.dram_tensor("b2", [D_MODEL], F32, kind="ExternalInput").ap(),
        "gate": nc.dram_tensor("gate", [T], F32, kind="ExternalInput").ap(),
        # yT[p, m, t] = y[t, m*P+p]
        "yT": nc.dram_tensor("yT", [P, M2, T], F32, kind="ExternalOutput").ap(),
    }
    with tile.TileContext(nc) as tc:
        _emit(tc, aps, T, mmdt, reps=reps, body_reps=body_reps)
    nc.compile()
    return nc


_NC_CACHE = {}


def _get_nc(T, mmdt=BF16):
    key = (T, mmdt)
    if key not in _NC_CACHE:
        _NC_CACHE[key] = build_nc(T, mmdt)
    return _NC_CACHE[key]


def _prearrange_w1(W1e, np_dt):
    # [D_MODEL, D_FF] -> [P, NG1, KM1, MG*P] with w1[p,g,k,j] = W1[k*P+p, g*MG*P+j]
    NG1 = M1 // MG
    return np.ascontiguousarray(
        W1e.reshape(KM1, P, NG1, MG * P).transpose(1, 2, 0, 3).astype(np_dt)
    )


def _prearrange_w2(W2e, np_dt):
    # [D_FF, D_MODEL] -> [P, M2, KM2, P] with w2[p,m,k,n] = W2[k*P+p, m*P+n]
    return np.ascontiguousarray(
        W2e.reshape(KM2, P, M2, P).transpose(1, 2, 0, 3).astype(np_dt)
    )


def shard_inputs(y_recv, x_flat, gate, local_eid, W1, b1, W2, b2, T_cap,
                 mm_np_dtype=ml_dtypes.bfloat16):
    """Gather tokens per expert, pad to T_cap, transpose. Returns in_maps + idx."""
    eid = np.asarray(local_eid).astype(np.int64)
    x_flat = np.asarray(x_flat)
    gate = np.asarray(gate)
    W1 = np.asarray(W1)
    W2 = np.asarray(W2)
    b1 = np.asarray(b1)
    b2 = np.asarray(b2)
    in_maps = []
    idxs = []
    for e in range(N_EXPERTS):
        idx = np.nonzero(eid == e)[0]
        idxs.append(idx)
        cnt = len(idx)
        # xT[p, k, t] = x[t, k*P+p]
        xT = np.zeros((P, KM1, T_cap), dtype=mm_np_dtype)
        xTfull = x_flat[idx].T.reshape(KM1, P, cnt).transpose(1, 0, 2)
        xT[:, :, :cnt] = xTfull.astype(mm_np_dtype)
        g = np.zeros((T_cap,), dtype=np.float32)
        g[:cnt] = gate[idx]
        in_maps.append(
            {
                "xT": xT,
                "w1": _prearrange_w1(W1[e], mm_np_dtype),
                "b1": np.ascontiguousarray(b1[e], dtype=np.float32),
                "w2": _prearrange_w2(W2[e], mm_np_dtype),
                "b2": np.ascontiguousarray(b2[e], dtype=np.float32),
                "gate": g,
            }
        )
    return in_maps, idxs


def t_cap_for(local_eid):
    eid = np.asarray(local_eid).astype(np.int64)
    counts = np.bincount(eid, minlength=N_EXPERTS)
    return max(256, int(-(-int(counts.max()) // 8) * 8))


def kernel(y_recv, x_flat, gate, local_eid, W1, b1, W2, b2, _trace=False):
    T_cap = t_cap_for(local_eid)

    in_maps, idxs = shard_inputs(y_recv, x_flat, gate, local_eid, W1, b1, W2, b2, T_cap)
    nc = _get_nc(T_cap)
    res = run_bass_kernel_spmd(
        nc, in_maps, core_ids=list(range(N_CORES)), trace=_trace
    )

    out = np.array(np.asarray(y_recv), dtype=np.float32, copy=True)
    for e in range(N_EXPERTS):
        idx = idxs[e]
        if len(idx):
            # yT[p, m, t] -> y[t, m*P+p]
            yT = res.results[e]["yT"]  # [P, M2, T_cap]
            y = yT.transpose(2, 1, 0).reshape(T_cap, D_MODEL)
            out[idx] = y[: len(idx)]
    if _trace:
        return out, res
    return out



# revision 2
# speedup vs baseline: 3.0264x; 3.0264x over previous
"""MoE expert-parallel FFN kernel for Trainium2 (8 NeuronCores).

Problem: 8192 tokens, d_model=768, d_ff=3072, 8 experts; each token is
routed (local_eid) to one expert's FFN: y = (relu(x@W1[e]+b1[e])@W2[e]+b2[e])*gate.

Sharding: expert parallelism - core e gets expert e's weights plus the
(gathered, transposed, zero-padded) tokens routed to expert e, computed in
transposed orientation (weights stationary, tokens moving):

    hT[d_ff, T]    = relu(W1.T @ xT + b1)
    yT[d_model, T] = (W2.T @ hT + b2) * gate

Mixed precision by gate magnitude: the final output is scaled by
gate in [0,1], so tokens with small gate tolerate proportionally larger FFN
error.  Per expert, the K8 lowest-gate tokens (gate < ~0.3) run the whole
FFN in fp8e4m3 with MatmulPerfMode.DoubleRow (K=256 contraction per
instruction -> 2x bf16 MAC rate; measured instr cost = max(~256, N) cycles),
the remaining T16 tokens run in bf16 (1 cycle/row).  Worst-case rel error of
the fp8 half is ~1.4e-2 against the 2e-2 gate; the bf16 half is ~3e-3.

fp8 scaling: weights are pre-scaled by 512 (keeps N(0,0.02) weights out of
the e4m3 subnormal range), h is kept as fp8(16*h).  The PSUM scale factors
are folded into the activation scale/bias (mm1) and the pre-scaled
bias/gate buffers (mm2 eviction), so no extra elementwise passes appear.

All weights are pre-arranged on the host so each per-m-tile weight DMA is a
single contiguous multi-KB-per-partition transfer. Accumulation is fp32 in
PSUM throughout.
"""

import numpy as np
import ml_dtypes

import concourse.bacc as bacc
import concourse.mybir as mybir
import concourse.tile as tile
from concourse.bass_utils import run_bass_kernel_spmd

P = 128
D_MODEL = 768
D_FF = 3072
N_EXPERTS = 8
N_CORES = 8
KM1 = D_MODEL // P   # 6  k-slabs for mm1
M1 = D_FF // P       # 24 m-tiles for mm1
KM2 = D_FF // P      # 24 k-slabs for mm2
M2 = D_MODEL // P    # 6  m-tiles for mm2
KP1 = KM1 // 2       # 3  DoubleRow k-pairs for mm1
KP2 = KM2 // 2       # 12 DoubleRow k-pairs for mm2
G_STAR = 0.30        # gate threshold for the fp8 path
WS = 512.0           # fp8 weight pre-scale
HS = 16.0            # fp8 h pre-scale
F32 = mybir.dt.float32
BF16 = mybir.dt.bfloat16
FP8 = mybir.dt.float8e4
DR = mybir.MatmulPerfMode.DoubleRow
NP_BF16 = ml_dtypes.bfloat16
NP_FP8 = ml_dtypes.float8_e4m3


def _split2(T):
    """Split T into two balanced chunks, each a multiple of 8."""
    a = (T // 2 + 7) // 8 * 8
    a = min(a, T)
    return [(0, a)] + ([(a, T)] if T > a else [])


def _emit_body(tc, aps, K8, T16, const_sb):
    nc = tc.nc
    Relu = mybir.ActivationFunctionType.Relu
    (b1q_sb, b1_sb, b2q_sb, b2_sb) = const_sb
    chs = _split2(T16)

    with (
        tc.tile_pool(name="xres", bufs=1) as xres,
        tc.tile_pool(name="hres", bufs=1) as hres,
        tc.tile_pool(name="w1s", bufs=2) as w1p,
        tc.tile_pool(name="w1qs", bufs=2) as w1qp,
        tc.tile_pool(name="w2s", bufs=2) as w2p,
        tc.tile_pool(name="w2qs", bufs=2) as w2qp,
        tc.tile_pool(name="ev", bufs=4) as evp,
        tc.tile_pool(name="ps", bufs=2, space="PSUM") as psp,
    ):
        # ---- per-iteration input loads ----
        xq_sb = xres.tile([P, KP1, 2, K8], FP8, tag="xq")
        nc.sync.dma_start(out=xq_sb[:], in_=aps["xq"])
        x16_sb = xres.tile([P, KM1, T16], BF16, tag="x16")
        nc.sync.dma_start(out=x16_sb[:], in_=aps["x16"])

        h8 = hres.tile([P, KP2, 2, K8], FP8, tag="h8")
        h16 = hres.tile([P, KM2, T16], BF16, tag="h16")

        # ---- mm1 ----
        for m in range(M1):
            wq = w1qp.tile([P, KP1, 2, P], FP8, tag="w1q")
            nc.sync.dma_start(out=wq[:], in_=aps["w1q"][:, m])
            wt = w1p.tile([P, KM1, P], BF16, tag="w1")
            nc.sync.dma_start(out=wt[:], in_=aps["w1"][:, m])

            ps8 = psp.tile([P, K8], F32, tag="p8", name="p8")
            for kp in range(KP1):
                nc.tensor.matmul(
                    ps8[:], lhsT=wq[:, kp], rhs=xq_sb[:, kp],
                    start=(kp == 0), stop=(kp == KP1 - 1), perf_mode=DR,
                )
            nc.scalar.activation(
                h8[:, m // 2, m % 2, :], ps8[:], Relu,
                bias=b1q_sb[:, m:m + 1], scale=HS / WS,
            )

            pst = [
                psp.tile([P, e - s], F32, tag=f"c{ci}", name=f"c{ci}")
                for ci, (s, e) in enumerate(chs)
            ]
            for k in range(KM1):
                for ci, (s, e) in enumerate(chs):
                    nc.tensor.matmul(
                        pst[ci][:], lhsT=wt[:, k], rhs=x16_sb[:, k, s:e],
                        start=(k == 0), stop=(k == KM1 - 1),
                    )
            for ci, (s, e) in enumerate(chs):
                nc.scalar.activation(
                    h16[:, m, s:e], pst[ci][:], Relu, bias=b1_sb[:, m:m + 1]
                )

        # ---- mm2 ----
        # gate buffers loaded late: only needed at eviction time.
        gate8_sb = xres.tile([P, K8], F32, tag="g8")
        nc.sync.dma_start(out=gate8_sb[:], in_=aps["gate8"].partition_broadcast(P))
        gate16_sb = xres.tile([P, T16], F32, tag="g16")
        nc.sync.dma_start(out=gate16_sb[:], in_=aps["gate16"].partition_broadcast(P))

        yT = aps["yT"]
        for m in range(M2):
            wq2 = w2qp.tile([P, KP2, 2, P], FP8, tag="w2q")
            nc.sync.dma_start(out=wq2[:], in_=aps["w2q"][:, m])
            wt2 = w2p.tile([P, KM2, P], BF16, tag="w2")
            nc.sync.dma_start(out=wt2[:], in_=aps["w2"][:, m])

            ps8 = psp.tile([P, K8], F32, tag="p8", name="p8")
            for kp in range(KP2):
                nc.tensor.matmul(
                    ps8[:], lhsT=wq2[:, kp], rhs=h8[:, kp],
                    start=(kp == 0), stop=(kp == KP2 - 1), perf_mode=DR,
                )
            yt8 = evp.tile([P, K8], F32, tag="y8")
            nc.vector.tensor_scalar_add(yt8[:], ps8[:], b2q_sb[:, m:m + 1])
            nc.vector.tensor_mul(yt8[:], yt8[:], gate8_sb[:])
            nc.sync.dma_start(out=yT[:, m, 0:K8], in_=yt8[:])

            pst = [
                psp.tile([P, e - s], F32, tag=f"c{ci}", name=f"c{ci}")
                for ci, (s, e) in enumerate(chs)
            ]
            for k in range(KM2):
                for ci, (s, e) in enumerate(chs):
                    nc.tensor.matmul(
                        pst[ci][:], lhsT=wt2[:, k], rhs=h16[:, k, s:e],
                        start=(k == 0), stop=(k == KM2 - 1),
                    )
            for ci, (s, e) in enumerate(chs):
                yt = evp.tile([P, e - s], F32, tag="y16")
                nc.vector.tensor_scalar_add(yt[:], pst[ci][:], b2_sb[:, m:m + 1])
                nc.vector.tensor_mul(yt[:], yt[:], gate16_sb[:, s:e])
                nc.sync.dma_start(out=yT[:, m, K8 + s:K8 + e], in_=yt[:])


def _emit(tc, aps, K8, T16, reps=1):
    nc = tc.nc
    with tc.tile_pool(name="const", bufs=1) as const:
        b1q_sb = const.tile([P, M1], F32)
        nc.sync.dma_start(out=b1q_sb[:], in_=aps["b1q"].rearrange("(m p) -> p m", p=P))
        b1_sb = const.tile([P, M1], F32)
        nc.sync.dma_start(out=b1_sb[:], in_=aps["b1"].rearrange("(m p) -> p m", p=P))
        b2q_sb = const.tile([P, M2], F32)
        nc.sync.dma_start(out=b2q_sb[:], in_=aps["b2q"].rearrange("(m p) -> p m", p=P))
        b2_sb = const.tile([P, M2], F32)
        nc.sync.dma_start(out=b2_sb[:], in_=aps["b2"].rearrange("(m p) -> p m", p=P))
        const_sb = (b1q_sb, b1_sb, b2q_sb, b2_sb)

        import contextlib
        loop_cm = tc.For_i(0, reps, 1) if reps != 1 else contextlib.nullcontext()
        with loop_cm:
            _emit_body(tc, aps, K8, T16, const_sb)


def build_nc(K8, T16, reps=1):
    nc = bacc.Bacc("TRN2", target_bir_lowering=False, debug=False)
    aps = {
        # xq[p, kp, i, t] = fp8(x[t, (2kp+i)*P + p]) for the K8 low-gate tokens
        "xq": nc.dram_tensor("xq", [P, KP1, 2, K8], FP8, kind="ExternalInput").ap(),
        # x16[p, k, t] = bf16(x[t, k*P + p]) for the T16 remaining tokens
        "x16": nc.dram_tensor("x16", [P, KM1, T16], BF16, kind="ExternalInput").ap(),
        # w1q[p, m, kp, i, j] = fp8(512*W1[(2kp+i)*P + p, m*P + j])
        "w1q": nc.dram_tensor("w1q", [P, M1, KP1, 2, P], FP8, kind="ExternalInput").ap(),
        # w1[p, m, k, j] = bf16(W1[k*P + p, m*P + j])
        "w1": nc.dram_tensor("w1", [P, M1, KM1, P], BF16, kind="ExternalInput").ap(),
        # w2q[p, m, kp, i, j] = fp8(512*W2[(2kp+i)*P + p, m*P + j])
        "w2q": nc.dram_tensor("w2q", [P, M2, KP2, 2, P], FP8, kind="ExternalInput").ap(),
        # w2[p, m, k, j] = bf16(W2[k*P + p, m*P + j])
        "w2": nc.dram_tensor("w2", [P, M2, KM2, P], BF16, kind="ExternalInput").ap(),
        "b1q": nc.dram_tensor("b1q", [D_FF], F32, kind="ExternalInput").ap(),
        "b1": nc.dram_tensor("b1", [D_FF], F32, kind="ExternalInput").ap(),
        "b2q": nc.dram_tensor("b2q", [D_MODEL], F32, kind="ExternalInput").ap(),
        "b2": nc.dram_tensor("b2", [D_MODEL], F32, kind="ExternalInput").ap(),
        "gate8": nc.dram_tensor("gate8", [K8], F32, kind="ExternalInput").ap(),
        "gate16": nc.dram_tensor("gate16", [T16], F32, kind="ExternalInput").ap(),
        # yT[p, m, t] = y[t, m*P + p], tokens ordered [K8 fp8 | T16 bf16]
        "yT": nc.dram_tensor("yT", [P, M2, K8 + T16], F32, kind="ExternalOutput").ap(),
    }
    with tile.TileContext(nc) as tc:
        _emit(tc, aps, K8, T16, reps=reps)
    nc.compile()
    return nc


_NC_CACHE = {}


def _get_nc(K8, T16):
    key = (K8, T16)
    if key not in _NC_CACHE:
        _NC_CACHE[key] = build_nc(K8, T16)
    return _NC_CACHE[key]


def plan_for(gate, local_eid):
    """(K8, T16) shapes shared by all cores (SPMD)."""
    gate = np.asarray(gate)
    eid = np.asarray(local_eid).astype(np.int64)
    counts = np.bincount(eid, minlength=N_EXPERTS)
    lows = np.array(
        [int((gate[eid == e] < G_STAR).sum()) for e in range(N_EXPERTS)]
    )
    K8 = int(lows.min()) // 8 * 8
    T16 = (int(counts.max()) - K8 + 7) // 8 * 8
    T16 = max(T16, 8)
    return K8, T16


def _arr_w_pair(W, kp, np_dt, scale):
    # [D, M*P] -> [P, M, KP, 2, P]; w[p,m,k2,i,j] = scale*W[(2*k2+i)*P+p, m*P+j]
    D, N = W.shape
    M = N // P
    r = (W * scale).reshape(kp, 2, P, M, P).transpose(2, 3, 0, 1, 4)
    return np.ascontiguousarray(r.astype(np_dt))


def _arr_w(W, np_dt):
    # [D, M*P] -> [P, M, K, P]; w[p,m,k,j] = W[k*P+p, m*P+j]
    D, N = W.shape
    K, M = D // P, N // P
    r = W.reshape(K, P, M, P).transpose(1, 2, 0, 3)
    return np.ascontiguousarray(r.astype(np_dt))


def shard_inputs(y_recv, x_flat, gate, local_eid, W1, b1, W2, b2, K8, T16):
    """Per-expert gather + gate split + transposes. Returns in_maps, idx lists."""
    eid = np.asarray(local_eid).astype(np.int64)
    x_flat = np.asarray(x_flat, dtype=np.float32)
    gate = np.asarray(gate, dtype=np.float32)
    W1 = np.asarray(W1)
    W2 = np.asarray(W2)
    b1 = np.asarray(b1)
    b2 = np.asarray(b2)
    in_maps = []
    idxs = []
    for e in range(N_EXPERTS):
        idx = np.nonzero(eid == e)[0]
        g = gate[idx]
        order = np.argsort(g, kind="stable")
        idx = idx[order]          # lowest-gate first
        idxs.append(idx)
        cnt = len(idx)
        n16 = cnt - K8

        xe = x_flat[idx]
        # fp8 part: first K8 tokens
        xq = np.ascontiguousarray(
            xe[:K8].T.reshape(KP1, 2, P, K8).transpose(2, 0, 1, 3).astype(NP_FP8)
        )
        # bf16 part
        x16 = np.zeros((P, KM1, T16), dtype=NP_BF16)
        x16[:, :, :n16] = (
            xe[K8:].T.reshape(KM1, P, n16).transpose(1, 0, 2).astype(NP_BF16)
        )
        g8 = np.ascontiguousarray(g[order][:K8] / (WS * HS), dtype=np.float32)
        g16 = np.zeros((T16,), dtype=np.float32)
        g16[:n16] = g[order][K8:]

        in_maps.append(
            {
                "xq": xq,
                "x16": x16,
                "w1q": _arr_w_pair(W1[e], KP1, NP_FP8, WS),
                "w1": _arr_w(W1[e], NP_BF16),
                "w2q": _arr_w_pair(W2[e], KP2, NP_FP8, WS),
                "w2": _arr_w(W2[e], NP_BF16),
                "b1q": np.ascontiguousarray(b1[e] * HS, dtype=np.float32),
                "b1": np.ascontiguousarray(b1[e], dtype=np.float32),
                "b2q": np.ascontiguousarray(b2[e] * (WS * HS), dtype=np.float32),
                "b2": np.ascontiguousarray(b2[e], dtype=np.float32),
                "gate8": g8,
                "gate16": g16,
            }
        )
    return in_maps, idxs


def kernel(y_recv, x_flat, gate, local_eid, W1, b1, W2, b2, _trace=False):
    K8, T16 = plan_for(gate, local_eid)
    in_maps, idxs = shard_inputs(
        y_recv, x_flat, gate, local_eid, W1, b1, W2, b2, K8, T16
    )
    nc = _get_nc(K8, T16)
    res = run_bass_kernel_spmd(
        nc, in_maps, core_ids=list(range(N_CORES)), trace=_trace
    )

    out = np.array(np.asarray(y_recv), dtype=np.float32, copy=True)
    for e in range(N_EXPERTS):
        idx = idxs[e]
        if len(idx):
            yT = res.results[e]["yT"]  # [P, M2, K8+T16]
            y = yT.transpose(2, 1, 0).reshape(K8 + T16, D_MODEL)
            out[idx] = y[: len(idx)]
    if _trace:
        return out, res
    return out
